# revision 2
# baseline (speedup 1.0000x reference)
"""LMClassifier forward (mean masked cross-entropy) on 8 Trainium2 cores.

Ragged-aware sharding: valid tokens (t < lens[b]-2) are compacted host-side
into one global list (masked positions never touch the device), split into
4 equal token-groups x 2 vocab-groups. Each core computes
  emb = sigmoid(ctx @ W1.T + b1)            (its tokens, all E)
  sumexp[tok] = sum_v exp((emb @ W2s.T + b2s) * inv_temp)   (its vocab shard)
  tgt_raw[tok] = emb . W2[tgt[tok]]          (ones-matmul partition reduce)
Host combines sumexp across vocab shards (logits are O(1), so no
max-subtraction is needed) and assembles the mean NLL over real tokens.
"""

import contextlib

import numpy as np
import ml_dtypes

import concourse.bacc as bacc
import concourse.tile as tile
import concourse.mybir as mybir
from concourse.bass_utils import run_bass_kernel_spmd

BF16 = mybir.dt.bfloat16
FP32 = mybir.dt.float32
AF = mybir.ActivationFunctionType


FP8 = mybir.dt.float8e4
FP8NP = mybir.dt.np(mybir.dt.float8e4)
W2_SCALE = 64.0  # keeps fp8-cast W2 out of the denormal range
W1_SCALE = 64.0  # same for W1; sigmoid's free affine divides it back out


class Cfg:
    def __init__(self, H, E, NT, VC, inv_temp=1.0, use_b2=False):
        assert H % 128 == 0 and E % 128 == 0 and NT % 128 == 0 and VC % 1024 == 0
        self.H, self.E, self.NT, self.VC = H, E, NT, VC
        self.inv_temp = float(inv_temp)
        self.use_b2 = use_b2
        self.n_k = H // 128    # contraction tiles for matmul1
        self.n_e = E // 128    # e-blocks (also contraction tiles for matmul2)
        self.n_sub = NT // 128 # token subblocks
        self.n_vp = VC // 1024 # vocab pairs (2 x 512)
        # token superblocks: 512-wide chunks with a 128-granular tail
        self.sbs = []
        t0 = 0
        while t0 < NT:
            w = min(512, NT - t0)
            self.sbs.append((t0, w))
            t0 += w
        assert self.n_e % 2 == 0 and self.n_k % 2 == 0


def build_lm_program(cfg):
    """Build the per-core SPMD Bass program. Returns compiled nc."""
    H, E, NT, VC = cfg.H, cfg.E, cfg.NT, cfg.VC
    nc = bacc.Bacc("TRN2", debug=False, target_bir_lowering=False)

    ctxT = nc.dram_tensor("ctxT", [H, NT], FP8, kind="ExternalInput").ap()
    w1t = nc.dram_tensor("w1t", [H, E], FP8, kind="ExternalInput").ap()
    b1 = nc.dram_tensor("b1", [E, 1], FP32, kind="ExternalInput").ap()
    w2t = nc.dram_tensor("w2t", [E, VC], FP8, kind="ExternalInput").ap()
    w2tgtT = nc.dram_tensor("w2tgtT", [E, NT], BF16, kind="ExternalInput").ap()
    ones_in = nc.dram_tensor("ones_in", [128, 1], BF16, kind="ExternalInput").ap()
    if cfg.use_b2:
        b2row = nc.dram_tensor("b2row", [1, VC], FP32, kind="ExternalInput").ap()
    sumexp_out = nc.dram_tensor(
        "sumexp_out", [128, cfg.n_sub], FP32, kind="ExternalOutput"
    ).ap()
    tgt_out = nc.dram_tensor("tgt_out", [1, NT], FP32, kind="ExternalOutput").ap()

    with contextlib.ExitStack() as ex:
        tc = ex.enter_context(tile.TileContext(nc))
        # persistent sbuf tensors
        const_pool = ex.enter_context(tc.tile_pool(name="const", bufs=1))
        w1_pool = ex.enter_context(tc.tile_pool(name="w1", bufs=1))
        emb_pool = ex.enter_context(tc.tile_pool(name="emb", bufs=1))
        acc_pool = ex.enter_context(tc.tile_pool(name="acc", bufs=1))
        # streamed tiles
        ctx_pool = ex.enter_context(tc.tile_pool(name="ctx", bufs=2))
        w2_pool = ex.enter_context(tc.tile_pool(name="w2", bufs=2))
        tgtw_pool = ex.enter_context(tc.tile_pool(name="tgtw", bufs=2))
        tmp_pool = ex.enter_context(tc.tile_pool(name="tmp", bufs=2))
        ps1_pool = ex.enter_context(tc.tile_pool(name="ps1", bufs=2, space="PSUM"))
        ps2_pool = ex.enter_context(tc.tile_pool(name="ps2", bufs=2, space="PSUM"))
        pst_pool = ex.enter_context(tc.tile_pool(name="pst", bufs=2, space="PSUM"))

        # ---- constants ----
        W1S = w1_pool.tile([128, cfg.n_k, E], FP8, tag="w1s")
        for k in range(cfg.n_k):
            eng = nc.sync if k % 2 == 0 else nc.scalar
            eng.dma_start(W1S[:, k : k + 1, :], w1t[k * 128 : (k + 1) * 128, :])
        B1S = const_pool.tile([128, cfg.n_e], FP32, tag="b1s")
        nc.sync.dma_start(B1S[:, :], b1.rearrange("(e p) one -> p (e one)", p=128))
        ONES = const_pool.tile([128, 1], BF16, tag="ones")
        nc.sync.dma_start(ONES[:, :], ones_in[:, :])
        if cfg.use_b2:
            B2S = const_pool.tile([1, VC], FP32, tag="b2s")
            nc.sync.dma_start(B2S[:, :], b2row[:, :])
            ONE1 = const_pool.tile([1, 128], FP32, tag="one1")
            nc.any.memset(ONE1[:, :], 1.0)

        EMB = emb_pool.tile([128, cfg.n_e, NT], BF16, tag="emb")
        EMB8 = emb_pool.tile([128, cfg.n_e, NT], FP8, tag="emb8")
        SUMP = acc_pool.tile([128, cfg.n_sub * cfg.n_vp], FP32, tag="sump")
        SOUT = acc_pool.tile([128, cfg.n_sub], FP32, tag="sout")
        TGT = acc_pool.tile([1, NT], FP32, tag="tgt")

        # ---- phase A: emb = sigmoid(W1 @ ctx + b1), [e, t] layout ----
        sig_scale = 1.0 / W1_SCALE
        w2_prefetch = {}
        for s, (t0, w) in enumerate(cfg.sbs):
            # stream the first two W2 shard tiles in under phase-A compute so
            # phase B starts without a DMA stall
            if s in (1, 2) and cfg.n_vp > 2 and len(cfg.sbs) > 2:
                vp = s - 1
                W2P = w2_pool.tile([128, cfg.n_e, 1024], FP8, tag="w2s")
                for e in range(cfg.n_e):
                    nc.sync.dma_start(
                        W2P[:, e : e + 1, :],
                        w2t[e * 128 : (e + 1) * 128, vp * 1024 : (vp + 1) * 1024],
                    )
                w2_prefetch[vp] = W2P
            CTXS = ctx_pool.tile([128, cfg.n_k, w], FP8, tag="ctxs")
            for k in range(cfg.n_k):
                eng = nc.scalar if (s == 0 and k % 2 == 0) else nc.sync
                eng.dma_start(
                    CTXS[:, k : k + 1, :],
                    ctxT[k * 128 : (k + 1) * 128, t0 : t0 + w],
                )
            for e in range(cfg.n_e):
                ps1 = ps1_pool.tile([128, w], FP32, tag="ps1")
                for kp in range(cfg.n_k // 2):
                    nc.tensor.matmul(
                        ps1[:, :],
                        W1S[:, 2 * kp : 2 * kp + 2, e * 128 : (e + 1) * 128],
                        CTXS[:, 2 * kp : 2 * kp + 2, :],
                        start=(kp == 0),
                        stop=(kp == cfg.n_k // 2 - 1),
                        perf_mode=mybir.MatmulPerfMode.DoubleRow,
                    )
                nc.scalar.activation(
                    EMB[:, e : e + 1, t0 : t0 + w],
                    ps1[:, :],
                    AF.Sigmoid,
                    bias=B1S[:, e : e + 1],
                    scale=sig_scale,
                )
                nc.scalar.activation(
                    EMB8[:, e : e + 1, t0 : t0 + w],
                    ps1[:, :],
                    AF.Sigmoid,
                    bias=B1S[:, e : e + 1],
                    scale=sig_scale,
                )

            # ---- phase A2: tgt_raw for this superblock ----
            TGW = tgtw_pool.tile([128, cfg.n_e, w], BF16, tag="tgw")
            nc.sync.dma_start(
                TGW[:, :, :],
                w2tgtT.rearrange("(e p) t -> p e t", p=128)[:, :, t0 : t0 + w],
            )
            pst = pst_pool.tile([1, w], FP32, tag="pst")
            for e in range(cfg.n_e):
                tmp = tmp_pool.tile([128, w], BF16, tag="tmp")
                nc.vector.tensor_mul(
                    tmp[:, :],
                    EMB[:, e, t0 : t0 + w],
                    TGW[:, e, :],
                )
                nc.tensor.matmul(
                    pst[:, :],
                    ONES[:, :],
                    tmp[:, :],
                    start=(e == 0),
                    stop=(e == cfg.n_e - 1),
                )
            nc.vector.tensor_copy(TGT[:, t0 : t0 + w], pst[:, :])

        # ---- phase B: logits, exp, accumulate ----
        exp_scale = cfg.inv_temp / W2_SCALE
        for vp in range(cfg.n_vp):
            if vp in w2_prefetch:
                W2S8 = w2_prefetch.pop(vp)
            else:
                W2S8 = w2_pool.tile([128, cfg.n_e, 1024], FP8, tag="w2s")
                for e in range(cfg.n_e):
                    nc.sync.dma_start(
                        W2S8[:, e : e + 1, :],
                        w2t[e * 128 : (e + 1) * 128, vp * 1024 : (vp + 1) * 1024],
                    )
            for sub in range(cfg.n_sub):
                ps2 = ps2_pool.tile([128, 1024], FP32, tag="ps2")
                for ep in range(cfg.n_e // 2):
                    lhsT = EMB8[:, 2 * ep : 2 * ep + 2, sub * 128 : (sub + 1) * 128]
                    for h in range(2):
                        nc.tensor.matmul(
                            ps2[:, h * 512 : (h + 1) * 512],
                            lhsT,
                            W2S8[:, 2 * ep : 2 * ep + 2, h * 512 : (h + 1) * 512],
                            start=(ep == 0),
                            stop=(ep == cfg.n_e // 2 - 1) and not cfg.use_b2,
                            perf_mode=mybir.MatmulPerfMode.DoubleRow,
                        )
                if cfg.use_b2:
                    for h in range(2):
                        nc.tensor.matmul(
                            ps2[:, h * 512 : (h + 1) * 512],
                            ONE1[:, :],
                            B2S[:, vp * 1024 + h * 512 : vp * 1024 + (h + 1) * 512],
                            start=False,
                            stop=True,
                        )
                nc.scalar.activation(
                    ps2[:, :],
                    ps2[:, :],
                    AF.Exp,
                    scale=exp_scale,
                    accum_out=SUMP[:, sub * cfg.n_vp + vp : sub * cfg.n_vp + vp + 1],
                )

        # ---- phase C: reduce partials, write outputs ----
        for sub in range(cfg.n_sub):
            nc.vector.reduce_sum(
                SOUT[:, sub : sub + 1],
                SUMP[:, sub * cfg.n_vp : (sub + 1) * cfg.n_vp],
                axis=mybir.AxisListType.X,
            )
        nc.sync.dma_start(sumexp_out[:, :], SOUT[:, :])
        nc.sync.dma_start(tgt_out[:, :], TGT[:, :])

    nc.compile()
    return nc


# ---------------- host side ----------------

T, B, H, E, V = 256, 32, 2048, 1024, 50257
NB, NV = 4, 2          # token-groups x vocab-groups
NVP = 51200            # padded vocab (NV * VC)
VC = NVP // NV


def _compact_tokens(lens):
    """Global valid-token list -> (per-sample counts, NT, total)."""
    lens = np.asarray(lens)
    cnt = np.clip(lens - 2, 0, T - 2).astype(np.int64)  # valid tokens per sample
    ntok = int(cnt.sum())
    nt = max(128, ((ntok + NB * 128 - 1) // (NB * 128)) * 128)
    return cnt, nt, ntok


def _shard_inputs(hidden, lens, token, W1, b1, W2):
    bf16 = ml_dtypes.bfloat16
    half = H // 2
    cnt, NT, ntok = _compact_tokens(lens)

    # compacted context rows [ntok, H] and targets [ntok]
    ctx_list = []
    tgt_list = []
    for b in range(B):
        c = int(cnt[b])
        if c == 0:
            continue
        ctx_list.append(
            np.concatenate(
                [hidden[:c, b, :half], hidden[2 : c + 2, b, half:]], axis=-1
            )
        )
        tgt_list.append(token[1 : c + 1, b])
    ctx_comp = np.concatenate(ctx_list, axis=0)  # [ntok, H] fp32
    tgt_comp = np.concatenate(tgt_list, axis=0)  # [ntok]

    W1T = np.ascontiguousarray(W1.T * W1_SCALE).astype(FP8NP)  # [H, E]
    W2T = np.zeros((E, NVP), dtype=FP8NP)
    W2T[:, :V] = (W2.T * W2_SCALE).astype(FP8NP)
    b1c = np.ascontiguousarray(b1.reshape(E, 1)).astype(np.float32)
    ones = np.ones((128, 1), dtype=bf16)

    in_maps = []
    for c in range(NB * NV):
        bg, vg = divmod(c, NV)
        lo = min(bg * NT, ntok)
        hi = min((bg + 1) * NT, ntok)
        n_real = hi - lo
        ctxT_c = np.zeros((H, NT), dtype=FP8NP)
        if n_real:
            ctxT_c[:, :n_real] = ctx_comp[lo:hi].T.astype(FP8NP)
        w2gT = np.zeros((E, NT), dtype=bf16)
        if n_real:
            w2gT[:, :n_real] = W2[tgt_comp[lo:hi], :].T.astype(bf16)
        in_maps.append(
            dict(
                ctxT=ctxT_c,
                w1t=W1T,
                b1=b1c,
                w2t=np.ascontiguousarray(W2T[:, vg * VC : (vg + 1) * VC]),
                w2tgtT=w2gT,
                ones_in=ones,
            )
        )
    return in_maps, tgt_comp, NT, ntok


def _combine(results, tgt_comp, NT, ntok, b2, inv_temp):
    """results: list of 8 dicts with sumexp_out [128, n_sub], tgt_out [1, NT]."""
    it = float(np.asarray(inv_temp).reshape(-1)[0])
    n_pad_v = NVP - V  # zero-padded vocab cols, all in the last shard
    b2 = np.asarray(b2, dtype=np.float64)

    total_nll = 0.0
    for bg in range(NB):
        lo = min(bg * NT, ntok)
        hi = min((bg + 1) * NT, ntok)
        n_real = hi - lo
        if n_real == 0:
            continue
        S = np.zeros(NT, dtype=np.float64)
        for vg in range(NV):
            r = results[bg * NV + vg]
            se = np.asarray(r["sumexp_out"], dtype=np.float64)  # [128, n_sub]
            S += se.T.reshape(NT)  # token n = sub*128 + p
            if vg == NV - 1:
                S -= n_pad_v  # exp(0)=1 per padded vocab column
        raw = np.asarray(results[bg * NV]["tgt_out"], dtype=np.float64).reshape(NT)
        logZ = np.log(S[:n_real])
        tgt_c = tgt_comp[lo:hi]
        logp_tgt = (raw[:n_real] + b2[tgt_c]) * it - logZ
        total_nll += -logp_tgt.sum()
    return np.float32(total_nll / ntok)


def kernel(hidden, lens, token, W1, b1, W2, b2, inv_temp):
    hidden = np.asarray(hidden, dtype=np.float32)
    lens = np.asarray(lens, dtype=np.int32)
    token = np.asarray(token, dtype=np.int32)
    W1 = np.asarray(W1, dtype=np.float32)
    b1 = np.asarray(b1, dtype=np.float32)
    W2 = np.asarray(W2, dtype=np.float32)
    b2 = np.asarray(b2, dtype=np.float32)
    inv_temp = np.asarray(inv_temp, dtype=np.float32)

    use_b2 = bool(np.any(b2 != 0.0))
    in_maps, tgt_comp, NT, ntok = _shard_inputs(hidden, lens, token, W1, b1, W2)
    cfg = Cfg(H, E, NT, VC, inv_temp=float(inv_temp.reshape(-1)[0]), use_b2=use_b2)
    nc = build_lm_program(cfg)
    if use_b2:
        b2p = np.zeros((1, NVP), dtype=np.float32)
        b2p[0, :V] = b2 * W2_SCALE
        for c in range(NB * NV):
            vg = c % NV
            in_maps[c]["b2row"] = np.ascontiguousarray(
                b2p[:, vg * VC : (vg + 1) * VC]
            )
    res = run_bass_kernel_spmd(nc, in_maps, core_ids=list(range(NB * NV)))
    return _combine(res.results, tgt_comp, NT, ntok, b2, inv_temp)


# revision 7
# speedup vs baseline: 1.0078x; 1.0078x over previous
"""LMClassifier forward (mean masked cross-entropy) on 8 Trainium2 cores.

Ragged-aware sharding: valid tokens (t < lens[b]-2) are compacted host-side
into one global list (masked positions never touch the device), split into
4 equal token-groups x 2 vocab-groups. Each core computes
  emb = sigmoid(ctx @ W1.T + b1)            (its tokens, all E; fp8 out)
  sumexp[tok] = sum_v exp((emb @ W2s.T + b2s) * inv_temp)   (its vocab shard)
and ships emb back. Host computes the target logit emb . W2[tgt] (cheap),
combines sumexp across vocab shards (logits are O(1), so no max-subtraction
is needed) and assembles the mean NLL over real tokens.
"""

import contextlib

import numpy as np
import ml_dtypes

import concourse.bacc as bacc
import concourse.tile as tile
import concourse.mybir as mybir
from concourse.bass_utils import run_bass_kernel_spmd

BF16 = mybir.dt.bfloat16
FP32 = mybir.dt.float32
AF = mybir.ActivationFunctionType


FP8 = mybir.dt.float8e4
FP8NP = mybir.dt.np(mybir.dt.float8e4)
W2_SCALE = 64.0  # keeps fp8-cast W2 out of the denormal range
W1_SCALE = 64.0  # same for W1; sigmoid's free affine divides it back out
VCHUNK = 2048    # vocab columns per PSUM tile (4 banks)


class Cfg:
    def __init__(self, H, E, NT, VC, inv_temp=1.0, use_b2=False):
        assert H % 128 == 0 and E % 128 == 0 and NT % 128 == 0 and VC % 1024 == 0
        self.H, self.E, self.NT, self.VC = H, E, NT, VC
        self.inv_temp = float(inv_temp)
        self.use_b2 = use_b2
        self.n_k = H // 128    # contraction tiles for matmul1
        self.n_e = E // 128    # e-blocks (also contraction tiles for matmul2)
        self.n_sub = NT // 128 # token subblocks
        # token superblocks: 512-wide chunks with a 128-granular tail
        self.sbs = []
        t0 = 0
        while t0 < NT:
            w = min(512, NT - t0)
            self.sbs.append((t0, w))
            t0 += w
        # vocab chunks: VCHUNK-wide with a 1024-granular tail
        self.vcs = []
        v0 = 0
        while v0 < VC:
            w = min(VCHUNK, VC - v0)
            self.vcs.append((v0, w))
            v0 += w
        self.n_vc = len(self.vcs)
        assert self.n_e % 2 == 0 and self.n_k % 2 == 0


def build_lm_program(cfg):
    """Build the per-core SPMD Bass program. Returns compiled nc."""
    H, E, NT, VC = cfg.H, cfg.E, cfg.NT, cfg.VC
    nc = bacc.Bacc("TRN2", debug=False, target_bir_lowering=False)

    ctxT = nc.dram_tensor("ctxT", [H, NT], FP8, kind="ExternalInput").ap()
    w1t = nc.dram_tensor("w1t", [H, E], FP8, kind="ExternalInput").ap()
    b1 = nc.dram_tensor("b1", [E, 1], FP32, kind="ExternalInput").ap()
    w2t = nc.dram_tensor("w2t", [E, VC], FP8, kind="ExternalInput").ap()
    if cfg.use_b2:
        b2row = nc.dram_tensor("b2row", [1, VC], FP32, kind="ExternalInput").ap()
    sumexp_out = nc.dram_tensor(
        "sumexp_out", [128, cfg.n_sub], FP32, kind="ExternalOutput"
    ).ap()
    emb_out = nc.dram_tensor(
        "emb_out", [128, cfg.n_e * NT], FP8, kind="ExternalOutput"
    ).ap()

    with contextlib.ExitStack() as ex:
        tc = ex.enter_context(tile.TileContext(nc))
        # persistent sbuf tensors
        const_pool = ex.enter_context(tc.tile_pool(name="const", bufs=1))
        w1_pool = ex.enter_context(tc.tile_pool(name="w1", bufs=1))
        emb_pool = ex.enter_context(tc.tile_pool(name="emb", bufs=1))
        acc_pool = ex.enter_context(tc.tile_pool(name="acc", bufs=1))
        # streamed tiles
        ctx_pool = ex.enter_context(tc.tile_pool(name="ctx", bufs=2))
        w2_pool = ex.enter_context(tc.tile_pool(name="w2", bufs=2))
        ps_pool = ex.enter_context(tc.tile_pool(name="ps", bufs=2, space="PSUM"))

        # ---- constants (sync queue; W2 stream lives on the vector queue) ----
        W1S = w1_pool.tile([128, cfg.n_k, E], FP8, tag="w1s")
        w1r = w1t.rearrange("(k p) e -> p k e", p=128)
        for kh in range(4):  # 4 pieces so matmul1 can start early
            nc.sync.dma_start(
                W1S[:, 4 * kh : 4 * kh + 4, :], w1r[:, 4 * kh : 4 * kh + 4, :]
            )
        B1S = const_pool.tile([128, cfg.n_e], FP32, tag="b1s")
        nc.sync.dma_start(B1S[:, :], b1.rearrange("(e p) one -> p (e one)", p=128))
        if cfg.use_b2:
            B2S = const_pool.tile([1, VC], FP32, tag="b2s")
            nc.gpsimd.dma_start(B2S[:, :], b2row[:, :])
            ONE1 = const_pool.tile([1, 128], FP32, tag="one1")
            nc.any.memset(ONE1[:, :], 1.0)

        EMB8 = emb_pool.tile([128, cfg.n_e, NT], FP8, tag="emb8")
        SUMP = acc_pool.tile([128, cfg.n_sub * cfg.n_vc], FP32, tag="sump")
        SOUT = acc_pool.tile([128, cfg.n_sub], FP32, tag="sout")

        # ---- phase A: emb = sigmoid(W1 @ ctx + b1), [e, t] layout ----
        sig_scale = 1.0 / W1_SCALE
        ctxr = ctxT.rearrange("(k p) t -> p k t", p=128)
        for s, (t0, w) in enumerate(cfg.sbs):
            CTXS = ctx_pool.tile([128, cfg.n_k, w], FP8, tag="ctxs",
                                 padded_shape=[128, cfg.n_k, 512])
            for kh in range(2):
                nc.sync.dma_start(
                    CTXS[:, 8 * kh : 8 * kh + 8, :],
                    ctxr[:, 8 * kh : 8 * kh + 8, t0 : t0 + w],
                )
            for e in range(cfg.n_e):
                ps1 = ps_pool.tile([128, w], FP32, tag="ps",
                                   padded_shape=[128, VCHUNK])
                for kp in range(cfg.n_k // 2):
                    nc.tensor.matmul(
                        ps1[:, :],
                        W1S[:, 2 * kp : 2 * kp + 2, e * 128 : (e + 1) * 128],
                        CTXS[:, 2 * kp : 2 * kp + 2, :],
                        start=(kp == 0),
                        stop=(kp == cfg.n_k // 2 - 1),
                        perf_mode=mybir.MatmulPerfMode.DoubleRow,
                    )
                nc.scalar.activation(
                    EMB8[:, e : e + 1, t0 : t0 + w],
                    ps1[:, :],
                    AF.Sigmoid,
                    bias=B1S[:, e : e + 1],
                    scale=sig_scale,
                )
        # ship emb to host for the target-logit dot (overlaps phase B)
        nc.sync.dma_start(
            emb_out.rearrange("p (e t) -> p e t", e=cfg.n_e), EMB8[:, :, :]
        )

        # ---- phase B: logits, exp, accumulate ----
        exp_scale = cfg.inv_temp / W2_SCALE
        w2r = w2t.rearrange("(e p) v -> p e v", p=128)
        for ci, (v0, vw) in enumerate(cfg.vcs):
            W2C = w2_pool.tile([128, cfg.n_e, vw], FP8, tag="w2c",
                               padded_shape=[128, cfg.n_e, VCHUNK])
            nc.gpsimd.dma_start(W2C[:, :, :], w2r[:, :, v0 : v0 + vw])
            nh = vw // 512
            for sub in range(cfg.n_sub):
                ps2 = ps_pool.tile([128, vw], FP32, tag="ps",
                                   padded_shape=[128, VCHUNK])
                for ep in range(cfg.n_e // 2):
                    prev = None
                    for h in range(nh):
                        mm = nc.tensor.matmul(
                            ps2[:, h * 512 : (h + 1) * 512],
                            EMB8[:, 2 * ep : 2 * ep + 2, sub * 128 : (sub + 1) * 128],
                            W2C[:, 2 * ep : 2 * ep + 2, h * 512 : (h + 1) * 512],
                            start=(ep == 0),
                            stop=(ep == cfg.n_e // 2 - 1) and not cfg.use_b2,
                            perf_mode=mybir.MatmulPerfMode.DoubleRow,
                        )
                        if h > 0:
                            # same stationary weights as h=0: skip the reload,
                            # pinned right after its loader in engine order
                            mm.ins.ldweights = False
                            mm.ins.add_dependency(
                                prev.ins.name,
                                mybir.DependencyInfo(sync=False, no_sync=True),
                            )
                        prev = mm
                if cfg.use_b2:
                    for h in range(nh):
                        nc.tensor.matmul(
                            ps2[:, h * 512 : (h + 1) * 512],
                            ONE1[:, :],
                            B2S[:, v0 + h * 512 : v0 + (h + 1) * 512],
                            start=False,
                            stop=True,
                        )
                nc.scalar.activation(
                    ps2[:, :],
                    ps2[:, :],
                    AF.Exp,
                    scale=exp_scale,
                    accum_out=SUMP[:, sub * cfg.n_vc + ci : sub * cfg.n_vc + ci + 1],
                )
                # drain per-sub on the last chunk so the tail is short
                if ci == cfg.n_vc - 1:
                    nc.vector.reduce_sum(
                        SOUT[:, sub : sub + 1],
                        SUMP[:, sub * cfg.n_vc : (sub + 1) * cfg.n_vc],
                        axis=mybir.AxisListType.X,
                    )
        nc.sync.dma_start(sumexp_out[:, :], SOUT[:, :])

    nc.compile()
    return nc


# ---------------- host side ----------------

T, B, H, E, V = 256, 32, 2048, 1024, 50257
NB, NV = 4, 2          # token-groups x vocab-groups
NVP = 51200            # padded vocab (NV * VC)
VC = NVP // NV


def _compact_tokens(lens):
    """Global valid-token list -> (per-sample counts, NT, total)."""
    lens = np.asarray(lens)
    cnt = np.clip(lens - 2, 0, T - 2).astype(np.int64)  # valid tokens per sample
    ntok = int(cnt.sum())
    nt = max(128, ((ntok + NB * 128 - 1) // (NB * 128)) * 128)
    return cnt, nt, ntok


def _shard_inputs(hidden, lens, token, W1, b1, W2):
    half = H // 2
    cnt, NT, ntok = _compact_tokens(lens)

    # compacted context rows [ntok, H] and targets [ntok]
    ctx_list = []
    tgt_list = []
    for b in range(B):
        c = int(cnt[b])
        if c == 0:
            continue
        ctx_list.append(
            np.concatenate(
                [hidden[:c, b, :half], hidden[2 : c + 2, b, half:]], axis=-1
            )
        )
        tgt_list.append(token[1 : c + 1, b])
    ctx_comp = np.concatenate(ctx_list, axis=0)  # [ntok, H] fp32
    tgt_comp = np.concatenate(tgt_list, axis=0)  # [ntok]

    W1T = np.ascontiguousarray(W1.T * W1_SCALE).astype(FP8NP)  # [H, E]
    W2T = np.zeros((E, NVP), dtype=FP8NP)
    W2T[:, :V] = (W2.T * W2_SCALE).astype(FP8NP)
    b1c = np.ascontiguousarray(b1.reshape(E, 1)).astype(np.float32)

    in_maps = []
    for c in range(NB * NV):
        bg, vg = divmod(c, NV)
        lo = min(bg * NT, ntok)
        hi = min((bg + 1) * NT, ntok)
        n_real = hi - lo
        ctxT_c = np.zeros((H, NT), dtype=FP8NP)
        if n_real:
            ctxT_c[:, :n_real] = ctx_comp[lo:hi].T.astype(FP8NP)
        in_maps.append(
            dict(
                ctxT=ctxT_c,
                w1t=W1T,
                b1=b1c,
                w2t=np.ascontiguousarray(W2T[:, vg * VC : (vg + 1) * VC]),
            )
        )
    return in_maps, tgt_comp, NT, ntok


def _combine(results, tgt_comp, NT, ntok, W2, b2, inv_temp):
    """results: 8 dicts with sumexp_out [128, n_sub], emb_out [128, n_e*NT]."""
    it = float(np.asarray(inv_temp).reshape(-1)[0])
    n_pad_v = NVP - V  # zero-padded vocab cols, all in the last shard
    b2 = np.asarray(b2, dtype=np.float64)
    n_e = E // 128

    total_nll = 0.0
    for bg in range(NB):
        lo = min(bg * NT, ntok)
        hi = min((bg + 1) * NT, ntok)
        n_real = hi - lo
        if n_real == 0:
            continue
        S = np.zeros(NT, dtype=np.float64)
        for vg in range(NV):
            r = results[bg * NV + vg]
            se = np.asarray(r["sumexp_out"], dtype=np.float64)  # [128, n_sub]
            S += se.T.reshape(NT)  # token n = sub*128 + p
            if vg == NV - 1:
                S -= n_pad_v  # exp(0)=1 per padded vocab column
        emb = (
            np.asarray(results[bg * NV]["emb_out"])
            .reshape(128, n_e, NT)
            .transpose(2, 1, 0)
            .reshape(NT, E)[:n_real]
            .astype(np.float32)
        )
        tgt_c = tgt_comp[lo:hi]
        raw = np.einsum("te,te->t", emb, W2[tgt_c, :], dtype=np.float64)
        logZ = np.log(S[:n_real])
        logp_tgt = (raw + b2[tgt_c]) * it - logZ
        total_nll += -logp_tgt.sum()
    return np.float32(total_nll / ntok)


def kernel(hidden, lens, token, W1, b1, W2, b2, inv_temp):
    hidden = np.asarray(hidden, dtype=np.float32)
    lens = np.asarray(lens, dtype=np.int32)
    token = np.asarray(token, dtype=np.int32)
    W1 = np.asarray(W1, dtype=np.float32)
    b1 = np.asarray(b1, dtype=np.float32)
    W2 = np.asarray(W2, dtype=np.float32)
    b2 = np.asarray(b2, dtype=np.float32)
    inv_temp = np.asarray(inv_temp, dtype=np.float32)

    use_b2 = bool(np.any(b2 != 0.0))
    in_maps, tgt_comp, NT, ntok = _shard_inputs(hidden, lens, token, W1, b1, W2)
    cfg = Cfg(H, E, NT, VC, inv_temp=float(inv_temp.reshape(-1)[0]), use_b2=use_b2)
    nc = build_lm_program(cfg)
    if use_b2:
        b2p = np.zeros((1, NVP), dtype=np.float32)
        b2p[0, :V] = b2 * W2_SCALE
        for c in range(NB * NV):
            vg = c % NV
            in_maps[c]["b2row"] = np.ascontiguousarray(
                b2p[:, vg * VC : (vg + 1) * VC]
            )
    res = run_bass_kernel_spmd(nc, in_maps, core_ids=list(range(NB * NV)))
    return _combine(res.results, tgt_comp, NT, ntok, W2, b2, inv_temp)
